# revision 3
# baseline (speedup 1.0000x reference)
"""CenterLoss kernel for Trainium2 (8 NeuronCores, data-parallel).

loss = sum((x - centers[labels])**2) / 2 / B

Strategy (per sharding hint): shard x/labels along batch across 8 cores,
replicate the small centers table, compute per-core partial sums on device,
sum the 8 scalars on host.

Per core (NS=8192 samples), per chunk of IC=1024 samples:
  - HWDGE DMA of the x chunk into SBUF as [128, T=8, 512] with
    tile[p, t, :] = x[c*IC + p*T + t, :]        (contiguous per partition)
  - gpsimd.dma_gather of the matching center rows from HBM. dma_gather
    writes gather-slot i to dst[i % 128, i // 128, :]; the host permutes
    the label order so slot i = t*128 + p corresponds to x row
    c*IC + p*T + t, making the two tiles elementwise-aligned.
  - DVE tensor_sub, ACT Square with accum_out -> acc[:, c]
Final: DVE reduce acc -> [128,1], PE matmul with ones -> [1,1] partial.
"""

import sys

sys.path.insert(0, "/opt/trn_rl_repo")

from contextlib import ExitStack

import numpy as np

import concourse.bass as bass  # noqa: F401  (AP types)
import concourse.tile as tile
from concourse import bacc, mybir
from concourse.bass_utils import run_bass_kernel_spmd

P = 128
D = 512
NCLASS = 1000
NCORES = 8
BATCH = 65536
NS = BATCH // NCORES  # 8192 samples per core


def build(ns: int = NS, ch: int = 8, num_devices: int = NCORES):
    """Build the per-core Bass program. ns samples / ch chunks per core."""
    ic = ns // ch  # samples per chunk
    t = ic // P  # free-dim tiles per chunk
    icols = ic // 16  # idx columns per chunk
    assert ic % 128 == 0 and ns % ch == 0

    nc = bacc.Bacc(
        "TRN2", target_bir_lowering=False, debug=False, num_devices=num_devices
    )
    x_d = nc.dram_tensor("x", [ns, D], mybir.dt.float32, kind="ExternalInput")
    idx_d = nc.dram_tensor(
        "idx", [P, ch * icols], mybir.dt.int16, kind="ExternalInput"
    )
    cen_d = nc.dram_tensor(
        "centers", [NCLASS, D], mybir.dt.float32, kind="ExternalInput"
    )
    out_d = nc.dram_tensor("out", [1, 1], mybir.dt.float32, kind="ExternalOutput")

    # x rows laid out so each partition reads one contiguous 2*T KiB run:
    # row = c*IC + p*T + t
    x_r = x_d.ap().rearrange("(c p t) d -> c p t d", c=ch, p=P)

    with tile.TileContext(nc) as tc, ExitStack() as ctx:
        const_pool = ctx.enter_context(tc.tile_pool(name="const", bufs=1))
        xp = ctx.enter_context(tc.tile_pool(name="xp", bufs=3))
        cp = ctx.enter_context(tc.tile_pool(name="cp", bufs=3))
        dp = ctx.enter_context(tc.tile_pool(name="dp", bufs=2))
        psp = ctx.enter_context(tc.tile_pool(name="psp", bufs=1, space="PSUM"))

        idx_sb = const_pool.tile([P, ch * icols], mybir.dt.int16)
        nc.sync.dma_start(idx_sb[:], idx_d.ap())
        acc = const_pool.tile([P, ch], mybir.dt.float32)

        for c in range(ch):
            xt = xp.tile([P, t, D], mybir.dt.float32)
            nc.sync.dma_start(xt[:], x_r[c])
            ct = cp.tile([P, t, D], mybir.dt.float32)
            nc.gpsimd.dma_gather(
                out_ap=ct[:],
                in_ap=cen_d.ap(),
                idxs_ap=idx_sb[:, c * icols : (c + 1) * icols],
                num_idxs=ic,
                num_idxs_reg=ic,
                elem_size=D,
            )
            df = dp.tile([P, t, D], mybir.dt.float32)
            nc.vector.tensor_sub(df[:], xt[:], ct[:])
            nc.scalar.activation(
                df[:],
                df[:],
                mybir.ActivationFunctionType.Square,
                accum_out=acc[:, c : c + 1],
            )

        red = const_pool.tile([P, 1], mybir.dt.float32)
        nc.vector.tensor_reduce(
            red[:], acc[:], axis=mybir.AxisListType.X, op=mybir.AluOpType.add
        )
        ones = const_pool.tile([P, 1], mybir.dt.float32)
        nc.gpsimd.memset(ones[:], 1.0)
        ps = psp.tile([1, 1], mybir.dt.float32)
        nc.tensor.matmul(ps[:], lhsT=red[:], rhs=ones[:], start=True, stop=True)
        res = const_pool.tile([1, 1], mybir.dt.float32)
        nc.vector.tensor_copy(res[:], ps[:])
        nc.sync.dma_start(out_d.ap(), res[:])

    nc.compile()
    return nc


def make_idx(labels_shard: np.ndarray, ch: int) -> np.ndarray:
    """int16 idx tensor [128, ch*ic/16] for dma_gather, slot-permuted so
    gather slot i = t*128+p maps to x row c*IC + p*T + t."""
    ns = labels_shard.shape[0]
    ic = ns // ch
    t = ic // P
    ls = labels_shard.reshape(ch, P, t)  # [c, p, t] = label of row c*IC + p*T + t
    sf = ls.transpose(0, 2, 1).reshape(ch, ic)  # sf[c, t*128+p] = ls[c, p, t]
    idx16 = sf.reshape(ch, ic // 16, 16).transpose(2, 0, 1)  # [16, ch, ic/16]
    full = np.tile(idx16, (8, 1, 1)).reshape(P, ch * (ic // 16))
    return np.ascontiguousarray(full.astype(np.int16))


_NC = None


def run(x, labels, centers, **spmd_kwargs):
    """Shard, execute on 8 cores, return (loss_scalar_f32, BassKernelResults)."""
    global _NC
    if _NC is None:
        _NC = build()
    ch = 8

    x = np.ascontiguousarray(np.asarray(x, dtype=np.float32))
    centers = np.ascontiguousarray(np.asarray(centers, dtype=np.float32))
    labels = np.asarray(labels).astype(np.int64)

    in_maps = []
    for core in range(NCORES):
        sl = slice(core * NS, (core + 1) * NS)
        in_maps.append(
            {
                "x": x[sl],
                "idx": make_idx(labels[sl], ch),
                "centers": centers,
            }
        )

    res = run_bass_kernel_spmd(_NC, in_maps, list(range(NCORES)), **spmd_kwargs)
    total = 0.0
    for core in range(NCORES):
        total += float(res.results[core]["out"][0, 0])
    loss = total / 2.0 / x.shape[0]
    return np.array(loss, dtype=np.float32), res


def kernel(x: np.ndarray, labels: np.ndarray, centers: np.ndarray) -> np.ndarray:
    loss, _ = run(x, labels, centers)
    return loss
